# revision 1
# baseline (speedup 1.0000x reference)
"""Trainium2 Bass kernel for MLM tied-weight readout:
    x = embed[ids]; logits = x @ W.T + b; p = softmax(logits); out = p @ W

Strategy (8 NeuronCores, vocab-parallel / tensor-parallel):
  - Host: embedding gather + bf16 cast/transposes (index prep + sharding).
  - Each core owns a 4000-row vocab shard of W/b (padded to 4096).
  - Device, per core, single pass over the vocab shard:
      stage A: L^T[v, m] = (W_c x^T)       (contract h, PE, bf16)
               U^T = exp(L^T + b_c)        (ScalarE, PSUM->SBUF bf16)
      stage B: Y[m, h] += U^T.T @ W_c      (contract v, PE, bf16)
               Z[m]    += U^T.T @ ones     (N=1 matmul, same stationary)
  - ReduceScatter (sum over cores, scatter over tokens) of [8192, 1025]
    partials; each core divides its token slice by Z and outputs it.
"""

import sys

sys.path.insert(0, "/opt/trn_rl_repo")

import functools

import ml_dtypes
import numpy as np

import concourse.bass as bass
import concourse.mybir as mybir
import concourse.tile as tile
from concourse import bacc
from concourse.bass_utils import run_bass_kernel_spmd

BF16 = mybir.dt.bfloat16
FP32 = mybir.dt.float32
FP8 = mybir.dt.float8e4

USE_FP8_A = True                # stage A (logits) in fp8e4m3 DoubleRow
USE_FP8_B = True                # stage B via u = U-1 in fp8e4m3 DoubleRow
FP8_SCALE = 16.0                # x,W pre-scale so values exit e4m3 denormals

B, T, H, V = 4, 2048, 1024, 32000
N_CORES = 8
V_SHARD = V // N_CORES          # 4000
V_PAD = 4096                    # padded shard (32 tiles of 128)
NV = V_PAD // 128               # 32 v-tiles per core
NK = H // 128                   # 8 k-tiles (hidden)
M_CHUNK = 512                   # tokens per stage-A chunk
PAD_BIAS = -30.0                # exp(-30) ~ 9e-14: padded rows contribute ~0


def build_program(n_tokens: int, with_rs: bool = True):
    """Build the SPMD Bass program for all 8 cores (same code, different data).

    with_rs=False builds a single-core variant (collective replaced by a DMA
    copy) for TimelineSim cost-model profiling.
    """
    n_chunks = n_tokens // M_CHUNK
    mt_per_chunk = M_CHUNK // 128
    tok_shard = n_tokens // N_CORES

    nc = bacc.Bacc(
        "TRN2",
        target_bir_lowering=False,
        debug=False,
        enable_asserts=False,
        num_devices=N_CORES if with_rs else 1,
    )

    a_dt = FP8 if USE_FP8_A else BF16
    b_dt = FP8 if USE_FP8_B else BF16
    xT = nc.dram_tensor("xT", [H, n_tokens], a_dt, kind="ExternalInput")
    WT = nc.dram_tensor("WT", [H, V_PAD], a_dt, kind="ExternalInput")
    Wn = nc.dram_tensor("Wn", [V_PAD, H], b_dt, kind="ExternalInput")
    bia = nc.dram_tensor("bia", [V_PAD, 1], FP32, kind="ExternalInput")
    if USE_FP8_B:
        csum = nc.dram_tensor("csum", [128, H], FP32, kind="ExternalInput")
    out = nc.dram_tensor("out", [tok_shard, H], FP32, kind="ExternalOutput")

    ypart = nc.dram_tensor("ypart", [n_tokens, H + 1], FP32)
    yrs = nc.dram_tensor("yrs", [tok_shard, H + 1], FP32)

    rg = [list(range(N_CORES))]
    # ReduceScatter is issued per row-group of RS_GROUP token rows so the
    # collective (TOPSP/SDMA) overlaps with the remaining PE compute. Rank c
    # receives rows [g*RS_GROUP + c*RS_OUT, +RS_OUT) -> out row block g.
    RS_GROUP = 1024
    RS_OUT = RS_GROUP // N_CORES                     # 128
    chunks_per_group = RS_GROUP // M_CHUNK           # 2

    with tile.TileContext(nc) as tc:
        with (
            tc.tile_pool(name="wn_res", bufs=1) as wn_pool,
            tc.tile_pool(name="wt_res", bufs=1) as wt_pool,
            tc.tile_pool(name="const", bufs=1) as const_pool,
            tc.tile_pool(name="xt", bufs=2) as xt_pool,
            tc.tile_pool(name="ut", bufs=1) as ut_pool,
            tc.tile_pool(name="usb", bufs=3) as usb_pool,
            tc.tile_pool(name="ysb", bufs=2) as ysb_pool,
            tc.tile_pool(name="zsb", bufs=2) as zsb_pool,
            tc.tile_pool(name="fin", bufs=2) as fin_pool,
            tc.tile_pool(name="psA", bufs=3, space="PSUM") as psA_pool,
            tc.tile_pool(name="psY", bufs=2, space="PSUM") as psY_pool,
            tc.tile_pool(name="psZ", bufs=1, space="PSUM") as psZ_pool,
        ):
            # --- resident weights (wt first: stage A needs it immediately) ---
            wt = []
            if USE_FP8_A:
                # DoubleRow: tile[p, i*V_PAD + v] = WT[k*256 + i*128 + p, v]
                for k in range(NK // 2):
                    t = wt_pool.tile([128, 2 * V_PAD], FP8, tag=f"wt{k}")
                    nc.sync.dma_start(t[:, 0:V_PAD], WT[k * 256 : k * 256 + 128, :])
                    nc.sync.dma_start(
                        t[:, V_PAD : 2 * V_PAD], WT[k * 256 + 128 : k * 256 + 256, :]
                    )
                    wt.append(t)
            else:
                for k in range(NK):
                    t = wt_pool.tile([128, V_PAD], BF16, tag=f"wt{k}")
                    nc.sync.dma_start(t[:], WT[k * 128 : (k + 1) * 128, :])
                    wt.append(t)
            wn = []
            if USE_FP8_B:
                # v-pair tiles: tile[p, i*H + h] = Wn[j*256 + i*128 + p, h]
                for j in range(NV // 2):
                    t = wn_pool.tile([128, 2 * H], FP8, tag=f"wn{j}")
                    nc.sync.dma_start(t[:, 0:H], Wn[j * 256 : j * 256 + 128, :])
                    nc.sync.dma_start(
                        t[:, H : 2 * H], Wn[j * 256 + 128 : j * 256 + 256, :]
                    )
                    wn.append(t)
            else:
                for v in range(NV):
                    t = wn_pool.tile([128, H], BF16, tag=f"wn{v}")
                    nc.sync.dma_start(t[:], Wn[v * 128 : (v + 1) * 128, :])
                    wn.append(t)
            btile = const_pool.tile([128, NV], FP32, tag="btile")
            for v in range(NV):
                nc.sync.dma_start(
                    btile[:, v : v + 1], bia[v * 128 : (v + 1) * 128, :]
                )
            ones = const_pool.tile([128, 2], FP8 if USE_FP8_B else BF16, tag="ones")
            nc.vector.memset(ones[:], 1.0)
            if USE_FP8_B:
                cs_tile = const_pool.tile([128, H], FP32, tag="cs")
                nc.sync.dma_start(cs_tile[:], csum[:])

            # --- main pipeline over token chunks ---
            for c in range(n_chunks):
                m0 = c * M_CHUNK
                xts = []
                if USE_FP8_A:
                    for k in range(NK // 2):
                        t = xt_pool.tile([128, 2 * M_CHUNK], FP8, tag=f"xt{k}")
                        nc.sync.dma_start(
                            t[:, 0:M_CHUNK],
                            xT[k * 256 : k * 256 + 128, m0 : m0 + M_CHUNK],
                        )
                        nc.sync.dma_start(
                            t[:, M_CHUNK : 2 * M_CHUNK],
                            xT[k * 256 + 128 : k * 256 + 256, m0 : m0 + M_CHUNK],
                        )
                        xts.append(t)
                else:
                    for k in range(NK):
                        t = xt_pool.tile([128, M_CHUNK], BF16, tag=f"xt{k}")
                        nc.sync.dma_start(
                            t[:], xT[k * 128 : (k + 1) * 128, m0 : m0 + M_CHUNK]
                        )
                        xts.append(t)
                # stage A: U^T[v, m] = exp(W_c x^T + b)
                ut = []
                for v in range(NV):
                    pA = psA_pool.tile([128, M_CHUNK], FP32, tag="pA")
                    if USE_FP8_A:
                        for k in range(NK // 2):
                            lhs3 = wt[k][:].rearrange("p (two v) -> p two v", two=2)
                            rhs3 = xts[k][:].rearrange("p (two m) -> p two m", two=2)
                            nc.tensor.matmul(
                                pA[:],
                                lhsT=lhs3[:, :, v * 128 : (v + 1) * 128],
                                rhs=rhs3,
                                start=(k == 0),
                                stop=(k == NK // 2 - 1),
                                perf_mode=mybir.MatmulPerfMode.DoubleRow,
                            )
                    else:
                        for k in range(NK):
                            nc.tensor.matmul(
                                pA[:],
                                lhsT=wt[k][:, v * 128 : (v + 1) * 128],
                                rhs=xts[k][:],
                                start=(k == 0),
                                stop=(k == NK - 1),
                            )
                    sA = (1.0 / (FP8_SCALE * FP8_SCALE)) if USE_FP8_A else 1.0
                    if USE_FP8_B:
                        # ACT emits S*exp(L+b) in fp32 (bias pre-folded with
                        # ln S on host); DVE subtracts S -> u8 = S*(U-1) fp8.
                        if v % 2 == 0:
                            up = ut_pool.tile([128, 2 * M_CHUNK], FP8, tag=f"ut{v // 2}")
                            ut.append(up)
                        usb = usb_pool.tile([128, M_CHUNK], FP32, tag="usb")
                        nc.scalar.activation(
                            usb[:],
                            pA[:],
                            mybir.ActivationFunctionType.Exp,
                            bias=btile[:, v : v + 1],
                            scale=sA,
                        )
                        half = v % 2
                        nc.vector.tensor_scalar_add(
                            ut[v // 2][:, half * M_CHUNK : (half + 1) * M_CHUNK],
                            usb[:],
                            -FP8_SCALE,
                        )
                    else:
                        u = ut_pool.tile([128, M_CHUNK], BF16, tag=f"ut{v}")
                        nc.scalar.activation(
                            u[:],
                            pA[:],
                            mybir.ActivationFunctionType.Exp,
                            bias=btile[:, v : v + 1],
                            scale=sA,
                        )
                        ut.append(u)
                # stage B: Y[m, h] = U W_c ; Z[m] = U ones
                for mt in range(mt_per_chunk):
                    pY = psY_pool.tile([128, H], FP32, tag="pY")
                    pZ = psZ_pool.tile([128, 1], FP32, tag="pZ")
                    if USE_FP8_B:
                        ones3 = ones[:].rearrange("p (two o) -> p two o", two=2)
                        for j in range(NV // 2):
                            lhs3 = ut[j][:].rearrange("p (two m) -> p two m", two=2)[
                                :, :, mt * 128 : (mt + 1) * 128
                            ]
                            rhs3 = wn[j][:].rearrange("p (two h) -> p two h", two=2)
                            st, sp = (j == 0), (j == NV // 2 - 1)
                            nc.tensor.matmul(
                                pY[:, 0:512], lhsT=lhs3, rhs=rhs3[:, :, 0:512],
                                start=st, stop=sp,
                                perf_mode=mybir.MatmulPerfMode.DoubleRow,
                            )
                            nc.tensor.matmul(
                                pY[:, 512:1024], lhsT=lhs3, rhs=rhs3[:, :, 512:1024],
                                start=st, stop=sp,
                                perf_mode=mybir.MatmulPerfMode.DoubleRow,
                            )
                            nc.tensor.matmul(
                                pZ[:], lhsT=lhs3, rhs=ones3,
                                start=st, stop=sp,
                                perf_mode=mybir.MatmulPerfMode.DoubleRow,
                            )
                    else:
                        for v in range(NV):
                            lhs = ut[v][:, mt * 128 : (mt + 1) * 128]
                            nc.tensor.matmul(
                                pY[:, 0:512], lhsT=lhs, rhs=wn[v][:, 0:512],
                                start=(v == 0), stop=(v == NV - 1),
                            )
                            nc.tensor.matmul(
                                pY[:, 512:1024], lhsT=lhs, rhs=wn[v][:, 512:1024],
                                start=(v == 0), stop=(v == NV - 1),
                            )
                            nc.tensor.matmul(
                                pZ[:], lhsT=lhs, rhs=ones[:, 0:1],
                                start=(v == 0), stop=(v == NV - 1),
                            )
                    ysb = ysb_pool.tile([128, H], FP32, tag="ysb")
                    nc.vector.tensor_copy(ysb[:], pY[:])
                    zsb = zsb_pool.tile([128, 1], FP32, tag="zsb")
                    nc.vector.tensor_copy(zsb[:], pZ[:])
                    r0 = m0 + mt * 128
                    nc.sync.dma_start(ypart[r0 : r0 + 128, 0:H], ysb[:])
                    nc.sync.dma_start(ypart[r0 : r0 + 128, H : H + 1], zsb[:])

                # --- per-row-group: reduce partials over cores + divide ---
                if (c + 1) % chunks_per_group == 0:
                    g = c // chunks_per_group
                    g0 = g * RS_GROUP
                    o0 = g * RS_OUT
                    if with_rs:
                        nc.gpsimd.collective_compute(
                            "ReduceScatter",
                            mybir.AluOpType.add,
                            replica_groups=rg,
                            ins=[ypart[g0 : g0 + RS_GROUP, :]],
                            outs=[yrs[o0 : o0 + RS_OUT, :]],
                        )
                    else:
                        nc.sync.dma_start(
                            yrs[o0 : o0 + RS_OUT, :], ypart[g0 : g0 + RS_OUT, :]
                        )
                    yt = fin_pool.tile([128, H + 1], FP32, tag="yt")
                    nc.sync.dma_start(yt[:], yrs[o0 : o0 + RS_OUT, :])
                    zinv = fin_pool.tile([128, 1], FP32, tag="zinv")
                    if USE_FP8_B:
                        # Z = zcol/S + S^2 cores... : zcol/S + V_PAD*N_CORES
                        zt = fin_pool.tile([128, 1], FP32, tag="zt")
                        nc.vector.tensor_scalar_mul(
                            zt[:], yt[:, H : H + 1], 1.0 / FP8_SCALE
                        )
                        zt2 = fin_pool.tile([128, 1], FP32, tag="zt2")
                        nc.vector.tensor_scalar_add(
                            zt2[:], zt[:], float(V_PAD * N_CORES)
                        )
                        nc.vector.reciprocal(zinv[:], zt2[:])
                        # numerator = Y/(S^2) + colsum(W)
                        ysc = fin_pool.tile([128, H], FP32, tag="ysc")
                        nc.vector.tensor_scalar_mul(
                            ysc[:], yt[:, 0:H], 1.0 / (FP8_SCALE * FP8_SCALE)
                        )
                        nc.vector.tensor_add(ysc[:], ysc[:], cs_tile[:])
                        num = ysc
                    else:
                        nc.vector.reciprocal(zinv[:], yt[:, H : H + 1])
                        num = None
                    ot = fin_pool.tile([128, H], FP32, tag="ot")
                    nc.scalar.mul(
                        ot[:], (num if num is not None else yt)[:, 0:H], mul=zinv[:, 0:1]
                    )
                    nc.sync.dma_start(out[o0 : o0 + RS_OUT, :], ot[:])

    nc.compile()
    return nc


@functools.lru_cache(maxsize=2)
def _cached_program(n_tokens: int):
    return build_program(n_tokens)


def prep_inputs(input_ids, embed_table, W, b, n_tokens=None):
    """Host-side sharding/prep: gather, cast to bf16, transpose, pad."""
    ids = np.asarray(input_ids).reshape(-1).astype(np.int64)
    if n_tokens is not None:
        ids = ids[:n_tokens]
    embed = np.ascontiguousarray(np.asarray(embed_table, dtype=np.float32))
    W = np.ascontiguousarray(np.asarray(W, dtype=np.float32))
    b = np.asarray(b, dtype=np.float32).reshape(-1)

    a_np = ml_dtypes.float8_e4m3 if USE_FP8_A else ml_dtypes.bfloat16
    a_scale = FP8_SCALE if USE_FP8_A else 1.0
    x = embed[ids]                                   # [n_tok, H] fp32
    xT = np.ascontiguousarray(x.T * a_scale).astype(a_np)      # [H, n_tok]

    b_np = ml_dtypes.float8_e4m3 if USE_FP8_B else ml_dtypes.bfloat16
    b_scale = FP8_SCALE if USE_FP8_B else 1.0
    csum = np.broadcast_to(W.sum(axis=0, dtype=np.float64).astype(np.float32), (128, H))
    csum = np.ascontiguousarray(csum)
    in_maps = []
    for c in range(N_CORES):
        lo = c * V_SHARD
        Wc = W[lo : lo + V_SHARD]                    # [4000, H]
        Wn_c = np.zeros((V_PAD, H), dtype=b_np)
        Wn_c[:V_SHARD] = (Wc * b_scale).astype(b_np)
        WT_c = np.zeros((H, V_PAD), dtype=a_np)
        WT_c[:, :V_SHARD] = np.ascontiguousarray(Wc.T * a_scale).astype(a_np)
        b_c = np.full((V_PAD, 1), PAD_BIAS, dtype=np.float32)
        b_c[:V_SHARD, 0] = b[lo : lo + V_SHARD]
        if USE_FP8_B:
            # ACT emits S*exp(L+b) directly: fold ln S into the bias
            b_c += np.log(FP8_SCALE)
        m = {"xT": xT, "WT": WT_c, "Wn": Wn_c, "bia": b_c}
        if USE_FP8_B:
            m["csum"] = csum
        in_maps.append(m)
    return in_maps


def run(inputs, n_tokens=B * T, **spmd_kwargs):
    nc = _cached_program(n_tokens)
    in_maps = prep_inputs(
        inputs["input_ids"], inputs["embed_table"], inputs["W"], inputs["b"],
        n_tokens=n_tokens,
    )
    res = run_bass_kernel_spmd(nc, in_maps, core_ids=list(range(N_CORES)), **spmd_kwargs)
    full = unshard([res.results[c]["out"] for c in range(N_CORES)], n_tokens)
    return full, res


def unshard(parts, n_tokens):
    # rank c's output rows are [g*1024 + c*128, +128) for each row-group g
    n_groups = n_tokens // 1024
    arr = np.stack([np.asarray(p).reshape(n_groups, 128, H) for p in parts], axis=1)
    return arr.reshape(n_tokens, H)                  # [n_tokens, H] fp32


def kernel(input_ids, embed_table, W, b):
    full, _ = run(
        {"input_ids": input_ids, "embed_table": embed_table, "W": W, "b": b}
    )
    return full.reshape(B, T, H).astype(np.float32)

